# revision 8
# baseline (speedup 1.0000x reference)
"""Cosformer (linear) attention kernel for 8 TRN2 NeuronCores.

Full (unsharded) inputs in, full output out.  Sharding: 8 cores =
4 batches x 2 head-halves.  Core c handles batch b = c//2 and heads
[hh*8, hh*8+8) where hh = c%2, i.e. embed cols [hh*512, (hh+1)*512).

Per-core math (all shapes per core):
  xT = x[:, b, :].T                        (E=1024, L=2048)  for q/k/v
  qT = relu(Wq_s @ x_q.T + bq_s)           [512, L]  (head dims on partitions)
  q2all[h] = [qT[h]*sin ; qT[h]*cos]       [128, L]  per head (partition-
             crossed DVE muls from qt_sb; sin/cos vary along L=free dim)
  k  = relu(x_k @ Wk_s.T + bk_s)           [L, 512]  (L on partitions)
  v  =      x_v @ Wv_s.T + bv_s            [L, 512]
  per head h (64 dims):
    k_ = [k*sin | k*cos]                   [L, 128]
    KV_aug = k_.T @ [v | 1]                [128, 66]   (col 64 = sum_l k_)
    o_aug = q2all[h].T @ KV_aug            [L, 66]
    o = o_aug[:, :64] / max(o_aug[:, 64], EPS)

Performance notes:
  - HAM clock gate: PE runs 1.2GHz cold / 2.4GHz after ~3.4us sustained
    activity; any >3.4us PE gap re-throttles.  Memset-fed warm-up matmuls
    bridge the ~8us DGE/DMA prologue; phases are back-to-back after that.
  - Aggregate DMA is ~390GB/s but shared by all queues, so bytes are
    issued in consumption order: q-phase inputs strictly before k/v-phase
    inputs.  All transfers are full DRAM rows (1-4KB packets; small
    packets are issue-bound at ~45ns each).
  - sin/cos [128, L] tables are generated on the PE (ones.T @ row) from
    two 4KB rows instead of DMAing 1MB.
  - DVE/GpSimd/ACT ops cost 0.2-0.5us fixed overhead each, so
    elementwise work is batched: q2 muls are full-L [64, 2048] ops, and
    the attention tail processes 4 L-chunks per iteration with a single
    batched z pipeline.
"""

import math
from contextlib import ExitStack

import numpy as np
import ml_dtypes

BF = ml_dtypes.bfloat16

import concourse.bass as bass
import concourse.bacc as bacc_mod
import concourse.mybir as mybir
from concourse.tile import TileContext
from concourse.bass_utils import run_bass_kernel_spmd

L = 2048            # sequence length
NB = 4              # batch
E = 1024            # embed dim
D = 64              # head dim
HC = 8              # heads per core
OC = HC * D         # 512 embed cols per core
P = 128
KC = E // P         # 8 contraction chunks over E
LC = L // P         # 16 L chunks of 128
NLC = L // 512      # 4 L chunks of 512
OCC = OC // P       # 4 q-proj output chunks
EPS = 1e-4
N_WARM = 10         # HAM warm-up matmuls bridging the DMA prologue

F32 = mybir.dt.float32
BF16 = mybir.dt.bfloat16
AF = mybir.ActivationFunctionType
MUL = mybir.AluOpType.mult


def build_nc(with_bias=True):
    nc = bacc_mod.Bacc()

    xq = nc.declare_dram_parameter("xq", [E, L], BF16, isOutput=False)
    xk = nc.declare_dram_parameter("xk", [E, L], BF16, isOutput=False)
    xv = nc.declare_dram_parameter("xv", [E, L], BF16, isOutput=False)
    wq = nc.declare_dram_parameter("wq", [E, OC], BF16, isOutput=False)
    wk = nc.declare_dram_parameter("wk", [E, OC], BF16, isOutput=False)
    wv = nc.declare_dram_parameter("wv", [E, OC], BF16, isOutput=False)
    bqc = nc.declare_dram_parameter("bqc", [P, OCC], F32, isOutput=False)
    bkr = nc.declare_dram_parameter("bkr", [1, OC], BF16, isOutput=False)
    bvr = nc.declare_dram_parameter("bvr", [1, OC], BF16, isOutput=False)
    sinr = nc.declare_dram_parameter("sinr", [1, L], BF16, isOutput=False)
    cosr = nc.declare_dram_parameter("cosr", [1, L], BF16, isOutput=False)
    sincol = nc.declare_dram_parameter("sincol", [P, LC], F32, isOutput=False)
    coscol = nc.declare_dram_parameter("coscol", [P, LC], F32, isOutput=False)
    outd = nc.declare_dram_parameter("out", [L, OC], BF16, isOutput=True)

    xq_r = xq.rearrange("(kc p) l -> p kc l", p=P)
    xk_r = xk.rearrange("(kc p) l -> p kc l", p=P)
    xv_r = xv.rearrange("(kc p) l -> p kc l", p=P)
    wq_r = wq.rearrange("(kc p) o -> p kc o", p=P)
    wk_r = wk.rearrange("(kc p) o -> p kc o", p=P)
    wv_r = wv.rearrange("(kc p) o -> p kc o", p=P)
    out_r4 = outd.rearrange("(g i p) o -> g p i o", p=P, i=4)

    with TileContext(nc) as tc, ExitStack() as ctx:
        const = ctx.enter_context(tc.tile_pool(name="const", bufs=1))
        persist = ctx.enter_context(tc.tile_pool(name="persist", bufs=1))

        wq_t = const.tile([P, KC, OC], BF16)
        wk_t = const.tile([P, KC, OC], BF16)
        wv_t = const.tile([P, KC, OC], BF16)
        xk_t = const.tile([P, KC, L], BF16)
        xv_t = const.tile([P, KC, L], BF16)
        bq_t = const.tile([P, OCC], F32)
        bk_t = const.tile([1, OC], BF16)
        bv_t = const.tile([1, OC], BF16)
        ones_t = const.tile([1, P], BF16)
        sinr_t = const.tile([1, L], BF16)
        cosr_t = const.tile([1, L], BF16)
        sinf_t = const.tile([P, L], BF16)
        cosf_t = const.tile([P, L], BF16)
        sc_t = const.tile([P, LC], F32)
        cc_t = const.tile([P, LC], F32)

        qt_sb = persist.tile([P, OCC, L], BF16)      # relu(q).T  [o-dim, oc, l]
        q2all = persist.tile([P, HC, L], BF16)       # [qT*sin ; qT*cos] per head
        kv_sb = persist.tile([P, HC, D + 2], BF16)   # per-head KV_aug

        # ---------------- phase Q: q projection (transposed) --------------
        with ExitStack() as pq_ctx:
            xqp = pq_ctx.enter_context(tc.tile_pool(name="xqp", bufs=1))
            wrm = pq_ctx.enter_context(tc.tile_pool(name="wrm", bufs=1))
            pqp = pq_ctx.enter_context(tc.tile_pool(name="pqp", bufs=3,
                                                    space="PSUM"))

            xq_t = xqp.tile([P, KC, L], BF16, name="xq_t")

            # memsets first: they run on DVE and must not queue behind the
            # DVE-issued DMA descriptors below.
            warm_t = wrm.tile([P, 512], BF16, name="warm_t")
            nc.vector.memset(warm_t[:, :], 0.0)
            nc.vector.memset(ones_t[:, :], 1.0)

            # --- DMA issue order == consumption order ---------------------
            # Each DMA_DIRECT2D occupies its issuing engine ~1-2us, so
            # issues go on engines whose compute work starts later: sync and
            # DVE carry phase-Q bytes (DVE's first real op is ~35us in),
            # GpSimd carries all phase-KV bytes (its first mul is later
            # still).  Scalar issues nothing: the sin/cos table evictions
            # and qt evictions need ACT immediately.
            nc.sync.dma_start(out=sinr_t, in_=sinr[:, :])
            nc.sync.dma_start(out=cosr_t, in_=cosr[:, :])
            nc.sync.dma_start(out=wq_t[:, :, :], in_=wq_r[:, :, :])
            for nlc in range(NLC):
                sl = slice(nlc * 512, (nlc + 1) * 512)
                q_ = nc.sync if nlc % 2 == 0 else nc.scalar
                q_.dma_start(out=xq_t[:, :, sl], in_=xq_r[:, :, sl])
            if with_bias:
                nc.scalar.dma_start(out=bq_t, in_=bqc[:, :])
            # phase KV bytes on the GpSimd queue, consumption order
            nc.gpsimd.dma_start(out=wk_t[:, :, :], in_=wk_r[:, :, :])
            nc.gpsimd.dma_start(out=wv_t[:, :, :], in_=wv_r[:, :, :])
            nc.gpsimd.dma_start(out=xk_t[:, :, 0:1024], in_=xk_r[:, :, 0:1024])
            nc.gpsimd.dma_start(out=xv_t[:, :, 0:1024], in_=xv_r[:, :, 0:1024])
            nc.gpsimd.dma_start(out=xk_t[:, :, 1024:2048],
                                in_=xk_r[:, :, 1024:2048])
            nc.gpsimd.dma_start(out=xv_t[:, :, 1024:2048],
                                in_=xv_r[:, :, 1024:2048])
            nc.gpsimd.dma_start(out=sc_t, in_=sincol[:, :])
            nc.gpsimd.dma_start(out=cc_t, in_=coscol[:, :])
            if with_bias:
                nc.gpsimd.dma_start(out=bk_t, in_=bkr[:, :])
                nc.gpsimd.dma_start(out=bv_t, in_=bvr[:, :])

            # HAM warm-up on memset data (no DMA dependency): covers the
            # ~8us DGE prologue before the first input bytes land.
            warm_ps = pqp.tile([P, 512], F32, tag="pq", name="warm_ps")
            for w in range(N_WARM):
                nc.tensor.matmul(warm_ps[:, :], warm_t[:, 0:P], warm_t[:, :],
                                 start=True, stop=True)

            # sin/cos [128, L] table generation: ones.T @ row on the PE
            for nlc in range(NLC):
                sl = slice(nlc * 512, (nlc + 1) * 512)
                ps_s = pqp.tile([P, 512], F32, tag="pq", name="ps_s")
                nc.tensor.matmul(ps_s[:, :], ones_t[:, :], sinr_t[:, sl],
                                 start=True, stop=True)
                nc.scalar.activation(sinf_t[:, sl], ps_s[:, :], AF.Copy)
                ps_c = pqp.tile([P, 512], F32, tag="pq", name="ps_c")
                nc.tensor.matmul(ps_c[:, :], ones_t[:, :], cosr_t[:, sl],
                                 start=True, stop=True)
                nc.scalar.activation(cosf_t[:, sl], ps_c[:, :], AF.Copy)

            for oc in range(OCC):
                for nlc in range(NLC):
                    sl = slice(nlc * 512, (nlc + 1) * 512)
                    pq_t = pqp.tile([P, 512], F32, tag="pq", name="pq_t")
                    for kc in range(KC):
                        nc.tensor.matmul(
                            pq_t[:, :],
                            (wq_t[:, kc, oc * P:(oc + 1) * P]),
                            (xq_t[:, kc, sl]),
                            start=(kc == 0), stop=(kc == KC - 1))
                    if with_bias:
                        nc.scalar.activation(qt_sb[:, oc, sl], pq_t[:, :],
                                             AF.Relu, bias=bq_t[:, oc:oc + 1])
                    else:
                        nc.scalar.activation(qt_sb[:, oc, sl], pq_t[:, :],
                                             AF.Relu)

        # ---------------- phase KV: k/v projections + KV accumulation -----
        with ExitStack() as p1:
            kscp = p1.enter_context(tc.tile_pool(name="kscp", bufs=3))
            vap = p1.enter_context(tc.tile_pool(name="vap", bufs=3))
            pkp = p1.enter_context(tc.tile_pool(name="pkp", bufs=2, space="PSUM"))
            pvp = p1.enter_context(tc.tile_pool(name="pvp", bufs=2, space="PSUM"))
            kvp = p1.enter_context(tc.tile_pool(name="kvp", bufs=1, space="PSUM"))

            kv_ps = [
                kvp.tile([P, 4, D + 2], F32, name="kv_ps0"),
                kvp.tile([P, 4, D + 2], F32, name="kv_ps1"),
            ]

            for lc in range(LC):
                lsl = slice(lc * P, (lc + 1) * P)
                pk_t = pkp.tile([P, OC], F32, tag="pk", name="pk_t")
                for kc in range(KC):
                    nc.tensor.matmul(pk_t[:, :], (xk_t[:, kc, lsl]),
                                     (wk_t[:, kc, :]),
                                     start=(kc == 0),
                                     stop=(not with_bias and kc == KC - 1))
                if with_bias:
                    nc.tensor.matmul(pk_t[:, :], (ones_t[:, :]), (bk_t[:, :]),
                                     start=False, stop=True)

                pv_t = pvp.tile([P, OC], F32, tag="pv", name="pv_t")
                for kc in range(KC):
                    nc.tensor.matmul(pv_t[:, :], (xv_t[:, kc, lsl]),
                                     (wv_t[:, kc, :]),
                                     start=(kc == 0),
                                     stop=(not with_bias and kc == KC - 1))
                if with_bias:
                    nc.tensor.matmul(pv_t[:, :], (ones_t[:, :]), (bv_t[:, :]),
                                     start=False, stop=True)

                # k_sc[p, h, 0, :] = relu(k)*sin_l ; k_sc[p, h, 1, :] = relu(k)*cos_l
                # (sin/cos >= 0 on (0, pi/2], so relu(k*s) == relu(k)*s)
                ksc_t = kscp.tile([P, HC, 2, D], BF16, tag="ksc", name="ksc_t")
                pk_v = pk_t.rearrange("p (h d) -> p h d", d=D)
                nc.scalar.activation(ksc_t[:, :, 0, :], pk_v, AF.Relu,
                                     scale=sc_t[:, lc:lc + 1])
                nc.scalar.activation(ksc_t[:, :, 1, :], pk_v, AF.Relu,
                                     scale=cc_t[:, lc:lc + 1])

                va_t = vap.tile([P, HC, D + 2], BF16, tag="va", name="va_t")
                pv_v = pv_t.rearrange("p (h d) -> p h d", d=D)
                nc.scalar.activation(va_t[:, :, D:D + 2], pv_v[:, :, 0:2],
                                     AF.Copy, bias=1.0, scale=0.0)
                nc.vector.tensor_copy(va_t[:, :, 0:D], pv_v)

                # KV_aug accumulation: 4 heads share one PSUM bank; only the
                # very first matmul into each bank uses start=True (clears
                # has_written bank-wide), everything else start=False so the
                # per-element has_written bits do the right thing.
                for h in range(HC):
                    nc.tensor.matmul(
                        kv_ps[h // 4][:, h % 4, :],
                        (ksc_t[:, h, :, :]),
                        (va_t[:, h, :]),
                        start=(lc == 0 and h % 4 == 0),
                        stop=(lc == LC - 1 and h % 4 == 3),
                    )

                # q2all build: one full-L [64, 2048] mul per lc iteration.
                # Crossed-partition writes go on DVE (verified on HW), the
                # straight ones on the otherwise-idle GpSimd engine.
                oc, j = divmod(lc, 4)
                h0, h1 = 2 * oc, 2 * oc + 1
                if j == 0:
                    nc.gpsimd.tensor_tensor(q2all[0:D, h0, :],
                                            qt_sb[0:D, oc, :],
                                            sinf_t[0:D, :], MUL)
                elif j == 1:
                    nc.vector.tensor_tensor(q2all[D:P, h0, :],
                                            qt_sb[0:D, oc, :],
                                            cosf_t[0:D, :], MUL)
                elif j == 2:
                    nc.vector.tensor_tensor(q2all[0:D, h1, :],
                                            qt_sb[D:P, oc, :],
                                            sinf_t[D:P, :], MUL)
                else:
                    nc.gpsimd.tensor_tensor(q2all[D:P, h1, :],
                                            qt_sb[D:P, oc, :],
                                            cosf_t[D:P, :], MUL)

            # evict KV accumulators to SBUF on ACT
            nc.scalar.activation(kv_sb[:, 0:4, :], kv_ps[0][:, :, :], AF.Copy)
            nc.scalar.activation(kv_sb[:, 4:8, :], kv_ps[1][:, :, :], AF.Copy)

        # ---------------- phase ATTN: attention output ---------------------
        with ExitStack() as p3:
            posp = p3.enter_context(tc.tile_pool(name="posp", bufs=2))
            osbp = p3.enter_context(tc.tile_pool(name="osbp", bufs=2))
            zp = p3.enter_context(tc.tile_pool(name="zp", bufs=2))
            pop = p3.enter_context(tc.tile_pool(name="pop", bufs=4, space="PSUM"))

            # 4 L-chunks per iteration: po psum raw-evicted to bf16 (split
            # ACT / DVE), then ONE batched z pipeline and two batched
            # normalize muls (DVE / GpSimd) per group, one out-DMA per group.
            for g in range(4):
                po4 = posp.tile([P, 4, HC, D + 2], BF16, tag="po4", name="po4")
                o4 = osbp.tile([P, 4, OC], BF16, tag="o4", name="o4")
                ov4 = o4.rearrange("p i (h d) -> p i h d", d=D)
                for i in range(4):
                    lc = 4 * g + i
                    for hg in range(2):
                        po_t = pop.tile([P, 4, D + 2], F32, tag="po",
                                        name="po_t")
                        for j in range(4):
                            h = hg * 4 + j
                            nc.tensor.matmul(po_t[:, j, :],
                                             (q2all[:, h, lc * P:(lc + 1) * P]),
                                             (kv_sb[:, h, :]),
                                             start=True, stop=True)
                        if hg == 0:
                            nc.scalar.activation(po4[:, i, 0:4, :],
                                                 po_t[:, :, :], AF.Copy)
                        else:
                            nc.vector.tensor_copy(po4[:, i, 4:8, :],
                                                  po_t[:, :, :])
                z_t = zp.tile([P, 4, HC], F32, tag="z", name="z_t")
                zr_t = zp.tile([P, 4, HC], BF16, tag="zr", name="zr_t")
                nc.gpsimd.tensor_scalar_max(z_t[:, :, :], po4[:, :, :, D], EPS)
                with nc.allow_low_precision(reason="z recip in bf16; z ~O(100), 0.4% rel err ok at 2e-2 gate"):
                    nc.vector.reciprocal(zr_t[:, :, :], z_t[:, :, :])
                zb0 = zr_t[:, :, 0:4].unsqueeze(3).broadcast_to((P, 4, 4, D))
                zb1 = zr_t[:, :, 4:8].unsqueeze(3).broadcast_to((P, 4, 4, D))
                nc.vector.tensor_tensor(ov4[:, :, 0:4, :],
                                        po4[:, :, 0:4, 0:D], zb0, MUL)
                nc.gpsimd.tensor_tensor(ov4[:, :, 4:8, :],
                                        po4[:, :, 4:8, 0:D], zb1, MUL)
                nc.sync.dma_start(out=out_r4[g], in_=o4[:, :, :])

    nc.compile()
    return nc


_NC = {}


def _get_nc(with_bias=True):
    if with_bias not in _NC:
        _NC[with_bias] = build_nc(with_bias)
    return _NC[with_bias]


def _host_constants():
    idx = (math.pi / 2.0) * (np.arange(L, dtype=np.float64) + 1.0) / float(L)
    sinv = np.sin(idx).astype(np.float32)
    cosv = np.cos(idx).astype(np.float32)
    return {
        "sinr": np.ascontiguousarray(sinv.reshape(1, L)).astype(BF),
        "cosr": np.ascontiguousarray(cosv.reshape(1, L)).astype(BF),
        "sincol": np.ascontiguousarray(sinv.reshape(LC, P).T),
        "coscol": np.ascontiguousarray(cosv.reshape(LC, P).T),
    }


def kernel(query, key, value, Wq, bq, Wk, bk, Wv, bv):
    query = np.asarray(query, np.float32)
    key = np.asarray(key, np.float32)
    value = np.asarray(value, np.float32)
    Wq = np.asarray(Wq, np.float32)
    Wk = np.asarray(Wk, np.float32)
    Wv = np.asarray(Wv, np.float32)
    bq = np.asarray(bq, np.float32)
    bk = np.asarray(bk, np.float32)
    bv = np.asarray(bv, np.float32)

    consts = _host_constants()
    in_maps = []
    for c in range(8):
        b, hh = divmod(c, 2)
        sl = slice(hh * OC, (hh + 1) * OC)
        in_maps.append({
            "xq": np.ascontiguousarray(query[:, b, :].T).astype(BF),
            "xk": np.ascontiguousarray(key[:, b, :].T).astype(BF),
            "xv": np.ascontiguousarray(value[:, b, :].T).astype(BF),
            "wq": np.ascontiguousarray(Wq[sl, :].T).astype(BF),
            "wk": np.ascontiguousarray(Wk[sl, :].T).astype(BF),
            "wv": np.ascontiguousarray(Wv[sl, :].T).astype(BF),
            "bqc": np.ascontiguousarray(bq[sl].reshape(OCC, P).T),
            "bkr": np.ascontiguousarray(bk[sl].reshape(1, OC)).astype(BF),
            "bvr": np.ascontiguousarray(bv[sl].reshape(1, OC)).astype(BF),
            **consts,
        })

    with_bias = bool(np.any(bk) or np.any(bv))
    res = run_bass_kernel_spmd(_get_nc(with_bias), in_maps,
                               core_ids=list(range(8))).results

    out = np.empty((L, NB, E), np.float32)
    for c in range(8):
        b, hh = divmod(c, 2)
        out[:, b, hh * OC:(hh + 1) * OC] = res[c]["out"].astype(np.float32)
    return out


if __name__ == "__main__":
    nc = build_nc()
    print("build OK")


# revision 13
# speedup vs baseline: 1.2623x; 1.2623x over previous
"""Cosformer (linear) attention kernel for 8 TRN2 NeuronCores.

Full (unsharded) inputs in, full output out.  Sharding: 8 cores =
4 batches x 2 head-halves.  Core c handles batch b = c//2 and heads
[hh*8, hh*8+8) where hh = c%2, i.e. embed cols [hh*512, (hh+1)*512).

Per-core math (all shapes per core):
  xT = x[:, b, :].T                        (E=1024, L=2048)  for q/k/v
  qT = relu(Wq_s @ x_q.T + bq_s)           [512, L]  (head dims on partitions)
  q2all[h] = [qT[h]*sin ; qT[h]*cos]       [128, L]  per head (partition-
             crossed DVE muls from qt_sb; sin/cos vary along L=free dim)
  k  = relu(x_k @ Wk_s.T + bk_s)           [L, 512]  (L on partitions)
  v  =      x_v @ Wv_s.T + bv_s            [L, 512]
  per head h (64 dims):
    k_ = [k*sin | k*cos]                   [L, 128]
    KV_aug = k_.T @ [v | 1]                [128, 66]   (col 64 = sum_l k_)
    o_aug = q2all[h].T @ KV_aug            [L, 66]
    o = o_aug[:, :64] / max(o_aug[:, 64], EPS)

Performance notes:
  - HAM clock gate: PE runs 1.2GHz cold / 2.4GHz after ~3.4us sustained
    activity; any >3.4us PE gap re-throttles.  Memset-fed warm-up matmuls
    bridge the ~8us DGE/DMA prologue; phases are back-to-back after that.
  - Aggregate DMA is ~390GB/s but shared by all queues, so bytes are
    issued in consumption order: q-phase inputs strictly before k/v-phase
    inputs.  All transfers are full DRAM rows (1-4KB packets; small
    packets are issue-bound at ~45ns each).
  - sin/cos [128, L] tables are generated on the PE (ones.T @ row) from
    two 4KB rows instead of DMAing 1MB.
  - DVE/GpSimd/ACT ops cost 0.2-0.5us fixed overhead each, so
    elementwise work is batched: q2 muls are full-L [64, 2048] ops, and
    the attention tail processes 4 L-chunks per iteration with a single
    batched z pipeline.
"""

import math
from contextlib import ExitStack

import numpy as np
import ml_dtypes

BF = ml_dtypes.bfloat16

import concourse.bass as bass
import concourse.bacc as bacc_mod
import concourse.mybir as mybir
from concourse.tile import TileContext
from concourse.bass_utils import run_bass_kernel_spmd

L = 2048            # sequence length
NB = 4              # batch
E = 1024            # embed dim
D = 64              # head dim
HC = 8              # heads per core
OC = HC * D         # 512 embed cols per core
P = 128
KC = E // P         # 8 contraction chunks over E
LC = L // P         # 16 L chunks of 128
NLC = L // 512      # 4 L chunks of 512
OCC = OC // P       # 4 q-proj output chunks
EPS = 1e-4
N_WARM = 14         # HAM warm-up matmuls bridging the DMA prologue

F32 = mybir.dt.float32
BF16 = mybir.dt.bfloat16
AF = mybir.ActivationFunctionType
MUL = mybir.AluOpType.mult


def build_nc(with_bias=True):
    nc = bacc_mod.Bacc()

    xq = nc.declare_dram_parameter("xq", [E, L], BF16, isOutput=False)
    xk = nc.declare_dram_parameter("xk", [E, L], BF16, isOutput=False)
    xv = nc.declare_dram_parameter("xv", [E, L], BF16, isOutput=False)
    wq = nc.declare_dram_parameter("wq", [E, OC], BF16, isOutput=False)
    wk = nc.declare_dram_parameter("wk", [E, OC], BF16, isOutput=False)
    wv = nc.declare_dram_parameter("wv", [E, OC], BF16, isOutput=False)
    bqc = nc.declare_dram_parameter("bqc", [P, OCC], F32, isOutput=False)
    bkr = nc.declare_dram_parameter("bkr", [1, OC], BF16, isOutput=False)
    bvr = nc.declare_dram_parameter("bvr", [1, OC], BF16, isOutput=False)
    sinr = nc.declare_dram_parameter("sinr", [1, L], BF16, isOutput=False)
    cosr = nc.declare_dram_parameter("cosr", [1, L], BF16, isOutput=False)
    sincol = nc.declare_dram_parameter("sincol", [P, LC], F32, isOutput=False)
    coscol = nc.declare_dram_parameter("coscol", [P, LC], F32, isOutput=False)
    outd = nc.declare_dram_parameter("out", [L, OC], BF16, isOutput=True)

    xq_r = xq.rearrange("(kc p) l -> p kc l", p=P)
    xk_r = xk.rearrange("(kc p) l -> p kc l", p=P)
    xv_r = xv.rearrange("(kc p) l -> p kc l", p=P)
    wq_r = wq.rearrange("(kc p) o -> p kc o", p=P)
    wk_r = wk.rearrange("(kc p) o -> p kc o", p=P)
    wv_r = wv.rearrange("(kc p) o -> p kc o", p=P)
    out_r4 = outd.rearrange("(g i p) o -> g p i o", p=P, i=4)

    with TileContext(nc) as tc, ExitStack() as ctx:
        const = ctx.enter_context(tc.tile_pool(name="const", bufs=1))
        persist = ctx.enter_context(tc.tile_pool(name="persist", bufs=1))

        wq_t = const.tile([P, KC, OC], BF16)
        wk_t = const.tile([P, KC, OC], BF16)
        wv_t = const.tile([P, KC, OC], BF16)
        xk_t = const.tile([P, KC, L], BF16)
        xv_t = const.tile([P, KC, L], BF16)
        bq_t = const.tile([P, OCC], F32)
        bk_t = const.tile([1, OC], BF16)
        bv_t = const.tile([1, OC], BF16)
        ones_t = const.tile([1, P], BF16)
        sinr_t = const.tile([1, L], BF16)
        cosr_t = const.tile([1, L], BF16)
        sinf_t = const.tile([P, L], BF16)
        cosf_t = const.tile([P, L], BF16)
        sc_t = const.tile([P, LC], F32)
        cc_t = const.tile([P, LC], F32)

        qt_sb = persist.tile([P, OCC, L], BF16)      # relu(q).T  [o-dim, oc, l]
        q2all = persist.tile([P, HC, L], BF16)       # [qT*sin ; qT*cos] per head
        kv_sb = persist.tile([P, HC, D + 2], BF16)   # per-head KV_aug

        # ---------------- phase Q: q projection (transposed) --------------
        with ExitStack() as pq_ctx:
            xqp = pq_ctx.enter_context(tc.tile_pool(name="xqp", bufs=1))
            wrm = pq_ctx.enter_context(tc.tile_pool(name="wrm", bufs=1))
            pqp = pq_ctx.enter_context(tc.tile_pool(name="pqp", bufs=3,
                                                    space="PSUM"))

            xq_t = xqp.tile([P, KC, L], BF16, name="xq_t")

            # memsets first: they run on DVE and must not queue behind the
            # DVE-issued DMA descriptors below.
            warm_t = wrm.tile([P, 512], BF16, name="warm_t")
            nc.vector.memset(warm_t[:, :], 0.0)
            nc.vector.memset(ones_t[:, :], 1.0)

            # --- DMA issue order == consumption order ---------------------
            # Queues round-robin per packet, so phase-Q bytes are spread
            # over all three HW queues and strictly precede phase-KV bytes.
            # Scalar gets exactly ONE early issue (wq) because its ACT work
            # (table/qt evictions) starts ~10us in; GpSimd's first compute
            # (q2 muls) is ~45us in, so it carries the phase-KV stream.
            nc.scalar.dma_start(out=wq_t[:, :, :], in_=wq_r[:, :, :])
            nc.sync.dma_start(out=sinr_t, in_=sinr[:, :])
            nc.sync.dma_start(out=cosr_t, in_=cosr[:, :])
            nc.sync.dma_start(out=xq_t[:, :, 0:512], in_=xq_r[:, :, 0:512])
            for nlc in range(1, NLC):
                sl = slice(nlc * 512, (nlc + 1) * 512)
                nc.gpsimd.dma_start(out=xq_t[:, :, sl], in_=xq_r[:, :, sl])
            if with_bias:
                nc.sync.dma_start(out=bq_t, in_=bqc[:, :])
            # phase KV bytes on the GpSimd queue, consumption order
            nc.gpsimd.dma_start(out=wk_t[:, :, :], in_=wk_r[:, :, :])
            nc.gpsimd.dma_start(out=wv_t[:, :, :], in_=wv_r[:, :, :])
            nc.gpsimd.dma_start(out=xk_t[:, :, 0:1024], in_=xk_r[:, :, 0:1024])
            nc.gpsimd.dma_start(out=xv_t[:, :, 0:1024], in_=xv_r[:, :, 0:1024])
            nc.gpsimd.dma_start(out=xk_t[:, :, 1024:2048],
                                in_=xk_r[:, :, 1024:2048])
            nc.gpsimd.dma_start(out=xv_t[:, :, 1024:2048],
                                in_=xv_r[:, :, 1024:2048])
            nc.gpsimd.dma_start(out=sc_t, in_=sincol[:, :])
            nc.gpsimd.dma_start(out=cc_t, in_=coscol[:, :])
            if with_bias:
                nc.gpsimd.dma_start(out=bk_t, in_=bkr[:, :])
                nc.gpsimd.dma_start(out=bv_t, in_=bvr[:, :])

            # HAM warm-up on memset data (no DMA dependency): covers the
            # ~8us DGE prologue before the first input bytes land.
            warm_ps = pqp.tile([P, 512], F32, tag="pq", name="warm_ps")
            for w in range(N_WARM):
                nc.tensor.matmul(warm_ps[:, :], warm_t[:, 0:P], warm_t[:, :],
                                 start=True, stop=True)

            # sin/cos [128, L] table generation: ones.T @ row on the PE
            for nlc in range(NLC):
                sl = slice(nlc * 512, (nlc + 1) * 512)
                ps_s = pqp.tile([P, 512], F32, tag="pq", name="ps_s")
                nc.tensor.matmul(ps_s[:, :], ones_t[:, :], sinr_t[:, sl],
                                 start=True, stop=True)
                nc.scalar.activation(sinf_t[:, sl], ps_s[:, :], AF.Copy)
                ps_c = pqp.tile([P, 512], F32, tag="pq", name="ps_c")
                nc.tensor.matmul(ps_c[:, :], ones_t[:, :], cosr_t[:, sl],
                                 start=True, stop=True)
                nc.scalar.activation(cosf_t[:, sl], ps_c[:, :], AF.Copy)

            for oc in range(OCC):
                for nlc in range(NLC):
                    sl = slice(nlc * 512, (nlc + 1) * 512)
                    pq_t = pqp.tile([P, 512], F32, tag="pq", name="pq_t")
                    for kc in range(KC):
                        nc.tensor.matmul(
                            pq_t[:, :],
                            (wq_t[:, kc, oc * P:(oc + 1) * P]),
                            (xq_t[:, kc, sl]),
                            start=(kc == 0), stop=(kc == KC - 1))
                    if with_bias:
                        nc.scalar.activation(qt_sb[:, oc, sl], pq_t[:, :],
                                             AF.Relu, bias=bq_t[:, oc:oc + 1])
                    else:
                        nc.scalar.activation(qt_sb[:, oc, sl], pq_t[:, :],
                                             AF.Relu)

        # ---------------- phase KV: k/v projections + KV accumulation -----
        with ExitStack() as p1:
            kscp = p1.enter_context(tc.tile_pool(name="kscp", bufs=3))
            vap = p1.enter_context(tc.tile_pool(name="vap", bufs=3))
            pkp = p1.enter_context(tc.tile_pool(name="pkp", bufs=2, space="PSUM"))
            pvp = p1.enter_context(tc.tile_pool(name="pvp", bufs=2, space="PSUM"))
            kvp = p1.enter_context(tc.tile_pool(name="kvp", bufs=1, space="PSUM"))

            kv_ps = [
                kvp.tile([P, 4, D + 2], F32, name="kv_ps0"),
                kvp.tile([P, 4, D + 2], F32, name="kv_ps1"),
            ]

            for lc in range(LC):
                lsl = slice(lc * P, (lc + 1) * P)
                pk_t = pkp.tile([P, OC], F32, tag="pk", name="pk_t")
                for kc in range(KC):
                    nc.tensor.matmul(pk_t[:, :], (xk_t[:, kc, lsl]),
                                     (wk_t[:, kc, :]),
                                     start=(kc == 0),
                                     stop=(not with_bias and kc == KC - 1))
                if with_bias:
                    nc.tensor.matmul(pk_t[:, :], (ones_t[:, :]), (bk_t[:, :]),
                                     start=False, stop=True)

                pv_t = pvp.tile([P, OC], F32, tag="pv", name="pv_t")
                for kc in range(KC):
                    nc.tensor.matmul(pv_t[:, :], (xv_t[:, kc, lsl]),
                                     (wv_t[:, kc, :]),
                                     start=(kc == 0),
                                     stop=(not with_bias and kc == KC - 1))
                if with_bias:
                    nc.tensor.matmul(pv_t[:, :], (ones_t[:, :]), (bv_t[:, :]),
                                     start=False, stop=True)

                # k_sc[p, h, 0, :] = relu(k)*sin_l ; k_sc[p, h, 1, :] = relu(k)*cos_l
                # (sin/cos >= 0 on (0, pi/2], so relu(k*s) == relu(k)*s)
                ksc_t = kscp.tile([P, HC, 2, D], BF16, tag="ksc", name="ksc_t")
                pk_v = pk_t.rearrange("p (h d) -> p h d", d=D)
                nc.scalar.activation(ksc_t[:, :, 0, :], pk_v, AF.Relu,
                                     scale=sc_t[:, lc:lc + 1])
                nc.scalar.activation(ksc_t[:, :, 1, :], pk_v, AF.Relu,
                                     scale=cc_t[:, lc:lc + 1])

                # va on ACT only: the DVE FIFO carries the ~5us q2 muls and
                # must never gate the kv matmuls through va.
                va_t = vap.tile([P, HC, D + 2], BF16, tag="va", name="va_t")
                pv_v = pv_t.rearrange("p (h d) -> p h d", d=D)
                nc.scalar.activation(va_t[:, :, D:D + 2], pv_v[:, :, 0:2],
                                     AF.Copy, bias=1.0, scale=0.0)
                nc.scalar.activation(va_t[:, :, 0:D], pv_v, AF.Copy)

                # KV_aug accumulation: 4 heads share one PSUM bank; only the
                # very first matmul into each bank uses start=True (clears
                # has_written bank-wide), everything else start=False so the
                # per-element has_written bits do the right thing.
                for h in range(HC):
                    nc.tensor.matmul(
                        kv_ps[h // 4][:, h % 4, :],
                        (ksc_t[:, h, :, :]),
                        (va_t[:, h, :]),
                        start=(lc == 0 and h % 4 == 0),
                        stop=(lc == LC - 1 and h % 4 == 3),
                    )

                # q2all build: one full-L [64, 2048] mul per lc iteration,
                # alternating DVE / GpSimd (each ~5us; 8 per engine fits
                # under the 62us PE-bound phase).
                oc, j = divmod(lc, 4)
                h0, h1 = 2 * oc, 2 * oc + 1
                if j == 0:
                    nc.gpsimd.tensor_tensor(q2all[0:D, h0, :],
                                            qt_sb[0:D, oc, :],
                                            sinf_t[0:D, :], MUL)
                elif j == 1:
                    nc.vector.tensor_tensor(q2all[D:P, h0, :],
                                            qt_sb[0:D, oc, :],
                                            cosf_t[0:D, :], MUL)
                elif j == 2:
                    nc.vector.tensor_tensor(q2all[0:D, h1, :],
                                            qt_sb[D:P, oc, :],
                                            sinf_t[D:P, :], MUL)
                else:
                    nc.gpsimd.tensor_tensor(q2all[D:P, h1, :],
                                            qt_sb[D:P, oc, :],
                                            cosf_t[D:P, :], MUL)

            # evict KV accumulators to SBUF on ACT
            nc.scalar.activation(kv_sb[:, 0:4, :], kv_ps[0][:, :, :], AF.Copy)
            nc.scalar.activation(kv_sb[:, 4:8, :], kv_ps[1][:, :, :], AF.Copy)

        # ---------------- phase ATTN: attention output ---------------------
        with ExitStack() as p3:
            posp = p3.enter_context(tc.tile_pool(name="posp", bufs=3))
            osbp = p3.enter_context(tc.tile_pool(name="osbp", bufs=2))
            zp = p3.enter_context(tc.tile_pool(name="zp", bufs=2))
            pop = p3.enter_context(tc.tile_pool(name="pop", bufs=4, space="PSUM"))
            pzp = p3.enter_context(tc.tile_pool(name="pzp", bufs=2, space="PSUM"))

            # The aug (denominator) column goes to its own tiny psum tile so
            # po is a CONTIGUOUS one-bank [128, 8, 64] tile -- strided
            # 66-wide APs fall off the DVE fast path.  z is batched per
            # 4-lc group; the normalize mult reads psum directly, split
            # DVE (hg0) / ACT-evict + GpSimd (hg1).
            for g in range(4):
                o4 = osbp.tile([P, 4, OC], BF16, tag="o4", name="o4")
                pz_t = pzp.tile([P, 4, HC, 2], F32, tag="pz", name="pz_t")
                po_ts = []
                for i in range(4):
                    lc = 4 * g + i
                    po_t = pop.tile([P, HC, D], F32, tag="po", name="po_t")
                    po_ts.append(po_t)
                    for h in range(HC):
                        nc.tensor.matmul(po_t[:, h, :],
                                         (q2all[:, h, lc * P:(lc + 1) * P]),
                                         (kv_sb[:, h, 0:D]),
                                         start=(h == 0), stop=(h == HC - 1))
                        nc.tensor.matmul(pz_t[:, i, h, :],
                                         (q2all[:, h, lc * P:(lc + 1) * P]),
                                         (kv_sb[:, h, D:D + 2]),
                                         start=(i == 0 and h == 0),
                                         stop=(i == 3 and h == HC - 1))
                z_t = zp.tile([P, 4, HC], F32, tag="z", name="z_t")
                zr_t = zp.tile([P, 4, HC], BF16, tag="zr", name="zr_t")
                nc.vector.tensor_scalar_max(z_t[:, :, :], pz_t[:, :, :, 0], EPS)
                with nc.allow_low_precision(reason="z recip in bf16; z ~O(100), 0.4% rel err ok at 2e-2 gate"):
                    nc.vector.reciprocal(zr_t[:, :, :], z_t[:, :, :])
                po4h = posp.tile([P, 4, 4, D], BF16, tag="po4h", name="po4h")
                for i in range(4):
                    ovi = o4[:, i, :].rearrange("p (h d) -> p h d", d=D)
                    zb0 = zr_t[:, i, 0:4].unsqueeze(2).broadcast_to((P, 4, D))
                    zb1 = zr_t[:, i, 4:8].unsqueeze(2).broadcast_to((P, 4, D))
                    nc.vector.tensor_tensor(ovi[:, 0:4, :],
                                            po_ts[i][:, 0:4, :], zb0, MUL)
                    nc.scalar.activation(po4h[:, i, :, :],
                                         po_ts[i][:, 4:8, :], AF.Copy)
                    nc.gpsimd.tensor_tensor(ovi[:, 4:8, :],
                                            po4h[:, i, :, :], zb1, MUL)
                nc.sync.dma_start(out=out_r4[g], in_=o4[:, :, :])

    nc.compile()
    return nc


_NC = {}


def _get_nc(with_bias=True):
    if with_bias not in _NC:
        _NC[with_bias] = build_nc(with_bias)
    return _NC[with_bias]


def _host_constants():
    idx = (math.pi / 2.0) * (np.arange(L, dtype=np.float64) + 1.0) / float(L)
    sinv = np.sin(idx).astype(np.float32)
    cosv = np.cos(idx).astype(np.float32)
    return {
        "sinr": np.ascontiguousarray(sinv.reshape(1, L)).astype(BF),
        "cosr": np.ascontiguousarray(cosv.reshape(1, L)).astype(BF),
        "sincol": np.ascontiguousarray(sinv.reshape(LC, P).T),
        "coscol": np.ascontiguousarray(cosv.reshape(LC, P).T),
    }


def kernel(query, key, value, Wq, bq, Wk, bk, Wv, bv):
    query = np.asarray(query, np.float32)
    key = np.asarray(key, np.float32)
    value = np.asarray(value, np.float32)
    Wq = np.asarray(Wq, np.float32)
    Wk = np.asarray(Wk, np.float32)
    Wv = np.asarray(Wv, np.float32)
    bq = np.asarray(bq, np.float32)
    bk = np.asarray(bk, np.float32)
    bv = np.asarray(bv, np.float32)

    consts = _host_constants()
    in_maps = []
    for c in range(8):
        b, hh = divmod(c, 2)
        sl = slice(hh * OC, (hh + 1) * OC)
        in_maps.append({
            "xq": np.ascontiguousarray(query[:, b, :].T).astype(BF),
            "xk": np.ascontiguousarray(key[:, b, :].T).astype(BF),
            "xv": np.ascontiguousarray(value[:, b, :].T).astype(BF),
            "wq": np.ascontiguousarray(Wq[sl, :].T).astype(BF),
            "wk": np.ascontiguousarray(Wk[sl, :].T).astype(BF),
            "wv": np.ascontiguousarray(Wv[sl, :].T).astype(BF),
            "bqc": np.ascontiguousarray(bq[sl].reshape(OCC, P).T),
            "bkr": np.ascontiguousarray(bk[sl].reshape(1, OC)).astype(BF),
            "bvr": np.ascontiguousarray(bv[sl].reshape(1, OC)).astype(BF),
            **consts,
        })

    with_bias = bool(np.any(bk) or np.any(bv))
    res = run_bass_kernel_spmd(_get_nc(with_bias), in_maps,
                               core_ids=list(range(8))).results

    out = np.empty((L, NB, E), np.float32)
    for c in range(8):
        b, hh = divmod(c, 2)
        out[:, b, hh * OC:(hh + 1) * OC] = res[c]["out"].astype(np.float32)
    return out


if __name__ == "__main__":
    nc = build_nc()
    print("build OK")
